# revision 8
# baseline (speedup 1.0000x reference)
"""Causal self-attention with RoPE for Trainium2, 8 NeuronCores.

Sharding: core c = (batch b = c//2, head-group g = c%2 of 8 heads).

Design (~336us on HW vs 709us for the v1 transpose-based kernel):
  - x is host-transposed to [C, T]; q,k are computed directly in
    [chan, tok] layout (W stationary, xT moving) so NO DMA transposes
    exist anywhere. v is computed in [tok, chan] layout (xT stationary,
    Wv moving) as the y-matmul needs it.
  - RoPE in [chan, tok] layout: partition-pair swap via SBUF->SBUF DMA,
    then full-width [128,512] bf16 tensor_tensor ops on DVE with
    host-prepped cos/sin tables [128, T] (sin sign baked per partition);
    the swap-dependent ops lag one block (software pipeline) so the
    swap-DMA latency never head-of-line-blocks the DVE queue.
  - exp runs on paired [128,1024] PSUM tiles (both head-halves of a key
    tile) to amortize the ~350-cycle ACT overhead; causal mask applied
    by multiplying the exp output by a 0/1 prefix mask on DVE.
  - et -> exp -> y software-pipelined by one tile so PE does not wait
    on the scalar engine (which is ~74% busy with the exps).
  - phase-1 of chunk J+1 and c_proj of earlier chunks are interleaved
    into attention of chunk J at i-step granularity; this keeps the PE
    dense so the HAM clock-gate stays at K=8/8 (2.4 GHz) instead of
    throttling to 4/8 - the single biggest effect on this kernel.
  - denominators batch-gathered per chunk into one [8,512] reciprocal
    (bf16: the 1/S error is a benign common factor per (head,query));
    per-g normalization on the last chunk shortens the serial tail.
  - c_proj in bf16 (fp32r matmuls measured ~2x slower per instruction).
"""

import numpy as np

B, T, C, H = 4, 2048, 1024, 16
HD = 64
HC = 8            # heads per core
NCORES = 8
PT = 128          # partition tile
TT = T // PT      # 16 t-tiles
QCW = 512         # q-chunk width
NQC = T // QCW    # 4
NKT = T // PT     # 16 key tiles
VW = HD + 1       # 65: v columns + ones column
CB = C // PT      # 8 c-chan blocks

_PROG_CACHE: dict = {}
_last_in_maps = None


def _build_program(sched):
    import concourse.bass as bass
    import concourse.tile as tile
    from concourse import bacc, mybir

    F32 = mybir.dt.float32
    F32R = mybir.dt.float32r
    BF16 = mybir.dt.bfloat16
    AT = mybir.ActivationFunctionType
    OP = mybir.AluOpType

    nc = bacc.Bacc("TRN2", target_bir_lowering=False, debug=False)

    x_d = nc.dram_tensor("x", [C, T], BF16, kind="ExternalInput")
    wqk_d = nc.dram_tensor("wqk", [C, 2 * QCW], BF16, kind="ExternalInput")
    wv_d = nc.dram_tensor("wv", [C, QCW], BF16, kind="ExternalInput")
    wp_d = nc.dram_tensor("wp", [QCW, C], BF16, kind="ExternalInput")
    cos_d = nc.dram_tensor("cosn", [PT, T], BF16, kind="ExternalInput")
    sin_d = nc.dram_tensor("sinn", [PT, T], BF16, kind="ExternalInput")
    cntb_d = nc.dram_tensor("cntb", [PT, T], F32, kind="ExternalInput")
    iota_d = nc.dram_tensor("iotas", [PT, NKT], F32, kind="ExternalInput")
    out_d = nc.dram_tensor("out", [T, C], F32, kind="ExternalOutput")

    with tile.TileContext(nc) as tc:
        with (
            tc.tile_pool(name="persist", bufs=1) as pp,
            tc.tile_pool(name="xtp", bufs=2) as xtp,
            tc.tile_pool(name="work", bufs=3) as wk,
            tc.tile_pool(name="ropep", bufs=3) as rp,
            tc.tile_pool(name="epool", bufs=4) as ep,
            tc.tile_pool(name="bpool", bufs=6) as bp,
            tc.tile_pool(name="ipool", bufs=3) as ip,
            tc.tile_pool(name="ycp", bufs=5) as ycp,
            tc.tile_pool(name="osb", bufs=2) as op_,
            tc.tile_pool(name="psA", bufs=2, space="PSUM") as psA,
            tc.tile_pool(name="psE", bufs=2, space="PSUM") as psE,
            tc.tile_pool(name="psY", bufs=2, space="PSUM") as psY,
        ):
            # ---------------- persistent tiles ----------------
            qT = [[pp.tile([PT, QCW], BF16, tag=f"qT{g}c{c}", name=f"qT{g}c{c}")
                   for c in range(NQC)] for g in range(4)]
            kT = [[pp.tile([PT, QCW], BF16, tag=f"kT{g}c{c}", name=f"kT{g}c{c}")
                   for c in range(NQC)] for g in range(4)]
            yTp = [[pp.tile([PT, QCW], BF16, tag=f"yTp{g}c{c}", name=f"yTp{g}c{c}")
                    for c in range(NQC)] for g in range(4)]
            vaug = [pp.tile([PT, 4 * HC * VW], BF16, tag=f"vaug{c}", name=f"vaug{c}")
                    for c in range(NQC)]
            cosT = pp.tile([PT, T], BF16, tag="cosT")
            sinT = pp.tile([PT, T], BF16, tag="sinT")
            cntb = pp.tile([PT, T], F32, tag="cntb")
            iotas = pp.tile([PT, NKT], F32, tag="iotas")
            ones = pp.tile([PT, 1], F32, tag="ones")
            w_sb = pp.tile([PT, CB * 2 * QCW], BF16, tag="w")
            wv_sb = pp.tile([PT, CB * QCW], BF16, tag="wv")
            wp_sb = pp.tile([PT, (QCW // PT) * C], BF16, tag="wp")

            # --- input DMAs: critical-path first ---
            for cb in range(CB):
                nc.sync.dma_start(
                    w_sb[:, cb * 2 * QCW:(cb + 1) * 2 * QCW],
                    wqk_d[cb * PT:(cb + 1) * PT, :],
                )
            nc.gpsimd.dma_start(
                wv_sb[:].rearrange("p (cb n) -> p cb n", n=QCW),
                wv_d[:].rearrange("(cb p) n -> p cb n", p=PT),
            )
            nc.gpsimd.dma_start(cosT[:], cos_d[:])
            nc.gpsimd.dma_start(sinT[:], sin_d[:])
            # noop
            nc.gpsimd.dma_start(cntb[:], cntb_d[:])
            nc.gpsimd.dma_start(iotas[:], iota_d[:])
            nc.sync.dma_start(
                wp_sb[:].rearrange("p (k n) -> p k n", n=C),
                wp_d[:].rearrange("(k p) n -> p k n", p=PT),
            )
            nc.vector.memset(ones[:], 1.0)
            ones_ap = ones[:]
            ones_rep = bass.AP(ones_ap.tensor, ones_ap.offset,
                               [ones_ap.ap[0], [0, 4], [0, HC]])
            for c in range(NQC):
                nc.vector.tensor_copy(
                    vaug[c][:].rearrange("p (t h c) -> p t h c", h=HC, c=VW)
                    [:, :, :, HD],
                    ones_rep,
                )

            # ---------------- phase-1 piece generators ----------------
            xts = {}
            pend_rope = []

            def flush_rope():
                while pend_rope:
                    swp, stg_c, cs_, dstT = pend_rope.pop(0)
                    tmp2 = rp.tile([PT, QCW], BF16, tag="tmp2")
                    nc.vector.tensor_tensor(tmp2[:], swp[:], sinT[:, cs_],
                                            OP.mult)
                    nc.vector.tensor_tensor(dstT[:], stg_c[:], tmp2[:],
                                            OP.add)

            def load_xt(c):
                xtc = xtp.tile([PT, CB * QCW], BF16, tag="xt")
                for hb in range(2):
                    nc.scalar.dma_start(
                        xtc[:, hb * 4 * QCW:(hb + 1) * 4 * QCW]
                        .rearrange("p (cb t) -> p cb t", t=QCW),
                        x_d[:].rearrange("(cb p) t -> p cb t", p=PT)
                        [:, 4 * hb:4 * (hb + 1), c * QCW:(c + 1) * QCW],
                    )
                xts[c] = xtc

            def p1_qk(c, m):
                """qkv block m of chunk c: 8 matmuls + rope chain."""
                cs = slice(c * QCW, (c + 1) * QCW)
                xv = xts[c][:].rearrange("p (cb t) -> p cb t", t=QCW)
                ps = psA.tile([PT, QCW], F32, tag="mm")
                for cb in range(CB):
                    nc.tensor.matmul(
                        ps[:],
                        w_sb[:, cb * 2 * QCW + m * PT:
                             cb * 2 * QCW + (m + 1) * PT],
                        xv[:, cb, :],
                        start=(cb == 0), stop=(cb == CB - 1),
                        skip_group_check=True,
                    )
                stg = rp.tile([PT, QCW], BF16, tag="stg")
                nc.vector.tensor_copy(stg[:], ps[:])
                swp = rp.tile([PT, QCW], BF16, tag="swp")
                for h in range(2):
                    b0 = h * 64
                    nc.sync.dma_start(swp[b0:b0 + 32, :],
                                      stg[b0 + 32:b0 + 64, :])
                    nc.sync.dma_start(swp[b0 + 32:b0 + 64, :],
                                      stg[b0:b0 + 32, :])
                dstT = qT[m][c] if m < 4 else kT[m - 4][c]
                tmp = rp.tile([PT, QCW], BF16, tag="tmp")
                nc.vector.tensor_tensor(tmp[:], stg[:], cosT[:, cs], OP.mult)
                pend_rope.append((swp, tmp, cs, dstT))
                if len(pend_rope) > 1:
                    swp_, stg_c, cs_, dstT_ = pend_rope.pop(0)
                    tmp2 = rp.tile([PT, QCW], BF16, tag="tmp2")
                    nc.vector.tensor_tensor(tmp2[:], swp_[:], sinT[:, cs_],
                                            OP.mult)
                    nc.vector.tensor_tensor(dstT_[:], stg_c[:], tmp2[:],
                                            OP.add)

            def p1_v(c, tt):
                """v t-tile tt of chunk c: 8 matmuls + vaug copy."""
                t = 4 * c + tt
                xv = xts[c][:].rearrange("p (cb t) -> p cb t", t=QCW)
                ps = psA.tile([PT, QCW], F32, tag="mm")
                for cb in range(CB):
                    nc.tensor.matmul(
                        ps[:],
                        xv[:, cb, tt * PT:(tt + 1) * PT],
                        wv_sb[:, cb * QCW:(cb + 1) * QCW],
                        start=(cb == 0), stop=(cb == CB - 1),
                        skip_group_check=True,
                    )
                nc.vector.tensor_copy(
                    vaug[c][:, tt * HC * VW:(tt + 1) * HC * VW]
                    .rearrange("p (h c) -> p h c", c=VW)[:, :, 0:HD],
                    ps[:].rearrange("p (h c) -> p h c", c=HD),
                )

            def p1_pieces(c):
                return ([(p1_qk, (c, m)) for m in range(8)]
                        + [(p1_v, (c, tt)) for tt in range(4)])

            # ---------------- phase 1: chunk 0 only, straight ----------------
            xtc0 = xtp.tile([PT, CB * QCW], BF16, tag="xt")
            for cb in range(CB):
                nc.scalar.dma_start(
                    xtc0[:, cb * QCW:(cb + 1) * QCW],
                    x_d[cb * PT:(cb + 1) * PT, 0:QCW],
                )
            xts[0] = xtc0
            load_xt(1)
            for m in range(8):
                p1_qk(0, m)
            for tt in range(4):
                p1_v(0, tt)
            flush_rope()

            # ---------------- phase 2+3: attention with c_proj interleave ---
            def cproj_piece(J, tt, n):
                """One c_proj output tile of chunk J."""
                t = 4 * J + tt
                ps = psA.tile([PT, QCW], F32, tag="mm")
                for k4 in range(QCW // PT):
                    nc.tensor.matmul(
                        ps[:],
                        yTp[k4][J][:, tt * PT:(tt + 1) * PT],
                        wp_sb[:, k4 * C + n * QCW:
                              k4 * C + (n + 1) * QCW],
                        start=(k4 == 0), stop=(k4 == QCW // PT - 1),
                        skip_group_check=True,
                    )
                o_sb = op_.tile([PT, QCW], F32, tag="osb")
                nc.vector.tensor_copy(o_sb[:], ps[:])
                nc.sync.dma_start(
                    out_d[t * PT:(t + 1) * PT,
                          n * QCW:(n + 1) * QCW], o_sb[:]
                )

            def cproj_pieces(J):
                return [(cproj_piece, (J, tt, n))
                        for tt in range(4) for n in range(C // QCW)]

            def norm_g(J, g, ycg, rsrc, ra, rb):
                invcA = ip.tile([64, QCW], BF16, tag="invc")
                invcB = ip.tile([64, QCW], BF16, tag="invc")
                r0A = ip.tile([1, QCW], BF16, tag="r0")
                r0B = ip.tile([1, QCW], BF16, tag="r0")
                nc.sync.dma_start(r0A[:], rsrc[ra:ra + 1, :])
                nc.sync.dma_start(r0B[:], rsrc[rb:rb + 1, :])
                nc.gpsimd.partition_broadcast(invcA[:], r0A[:])
                nc.gpsimd.partition_broadcast(invcB[:], r0B[:])
                nc.vector.tensor_tensor(
                    yTp[g][J][0:HD, :], ycg[0:HD, 0:QCW], invcA[:], OP.mult
                )
                ytmp = wk.tile([64, QCW], BF16, tag="ytmp")
                nc.vector.tensor_tensor(
                    ytmp[:], ycg[0:HD, QCW:2 * QCW], invcB[:], OP.mult
                )
                nc.sync.dma_start(yTp[g][J][HD:2 * HD, :], ytmp[:])

            for J in range(NQC):
                qs = slice(J * QCW, (J + 1) * QCW)
                # deferred PE work interleaved into this chunk's attention,
                # balanced so late (scalar-heavy) chunks get filler too
                if J == 0:
                    pieces = p1_pieces(1)
                elif J == 1:
                    pieces = p1_pieces(2) + p1_pieces(3)[:6]
                elif J == 2:
                    pieces = p1_pieces(3)[6:]
                else:
                    pieces = (cproj_pieces(0) + cproj_pieces(1)
                              + cproj_pieces(2))
                if J + 2 < NQC:
                    load_xt(J + 2)
                nsteps = len(sched[J]) * 4
                npieces = len(pieces)
                step = 0
                emitted = 0
                bts = {}
                for (i, lo, hi) in sched[J]:
                    if hi > lo:
                        bt = bp.tile([PT, QCW], BF16, tag="B")
                        nc.vector.tensor_scalar(
                            bt[:, 0:hi - lo],
                            cntb[:, J * QCW + lo:J * QCW + hi],
                            iotas[:, i:i + 1], None, OP.is_gt,
                        )
                        bts[i] = bt
                stage8 = wk.tile([8, QCW], BF16, tag="st8")
                ycgs = []
                for g in range(4):
                    yA = psY.tile([VW, QCW], F32, tag="y")
                    yB = psY.tile([VW, QCW], F32, tag="y")
                    last = sched[J][-1][0]
                    pendq = []  # software pipeline: y lags et by two tiles

                    def emit_y(p):
                        (pi, plo, pE) = p
                        for hh in range(2):
                            h = 2 * g + hh
                            vc, vk = pi // 4, pi % 4
                            vcol = vk * HC * VW + h * VW
                            ybank = yA if hh == 0 else yB
                            nc.tensor.matmul(
                                ybank[:, plo:QCW],
                                vaug[vc][:, vcol:vcol + VW],
                                pE[:, hh * QCW + plo:(hh + 1) * QCW],
                                start=(pi == sched[J][0][0]), stop=(pi == last),
                                skip_group_check=True,
                            )

                    for (i, lo, hi) in sched[J]:
                        ks = slice(i * PT, (i + 1) * PT)
                        qr = slice(J * QCW + lo, (J + 1) * QCW)
                        et = psE.tile([PT, 2 * QCW], F32, tag="et")
                        bnd = i in bts
                        kc, kk = i // 4, i % 4
                        for hh in range(2):
                            base = 64 * hh
                            nc.tensor.matmul(
                                et[:, hh * QCW + lo:(hh + 1) * QCW],
                                kT[g][kc][base:base + HD,
                                          kk * PT:(kk + 1) * PT],
                                qT[g][J][base:base + HD, lo:QCW],
                                start=True, stop=True,
                                skip_group_check=True,
                            )
                        e_sb = ep.tile([PT, 2 * QCW], BF16, tag="E")
                        ev = e_sb[:].rearrange("p (two n) -> p two n", two=2)
                        pv = et[:].rearrange("p (two n) -> p two n", two=2)
                        nc.scalar.activation(
                            ev[:, :, lo:QCW], pv[:, :, lo:QCW], AT.Exp,
                            scale=0.125,
                        )
                        if bnd:
                            btap = bts[i][:, 0:hi - lo]
                            btrep = bass.AP(btap.tensor, btap.offset,
                                            [btap.ap[0], [0, 2], btap.ap[1]])
                            nc.vector.tensor_tensor(
                                ev[:, :, lo:hi], ev[:, :, lo:hi], btrep,
                                OP.mult,
                            )
                        pendq.append((i, lo, e_sb))
                        if len(pendq) > 2:
                            emit_y(pendq.pop(0))
                        step += 1
                        while emitted * nsteps < step * npieces:
                            fn, args = pieces[emitted]
                            fn(*args)
                            emitted += 1
                    while pendq:
                        emit_y(pendq.pop(0))
                    # drain + denominator gather
                    ycg = ycp.tile([VW, 2 * QCW], BF16, tag="ycop")
                    nc.vector.tensor_copy(ycg[0:VW, 0:QCW], yA[:])
                    nc.vector.tensor_copy(ycg[0:VW, QCW:2 * QCW], yB[:])
                    if J == NQC - 1:
                        # per-g normalization: keeps the final-chunk tail
                        # chain short (only g=3's chain is serial at the end)
                        st2 = wk.tile([2, QCW], BF16, tag="st8")
                        nc.sync.dma_start(st2[0:1, :], ycg[64:65, 0:QCW])
                        nc.sync.dma_start(st2[1:2, :], ycg[64:65, QCW:2 * QCW])
                        rc2 = wk.tile([2, QCW], BF16, tag="recip")
                        with nc.allow_low_precision(reason="bf16 denom"):
                            nc.vector.reciprocal(rc2[:], st2[:])
                        norm_g(J, g, ycg, rc2, 0, 1)
                    else:
                        nc.sync.dma_start(stage8[2 * g:2 * g + 1, :],
                                          ycg[64:65, 0:QCW])
                        nc.sync.dma_start(stage8[2 * g + 1:2 * g + 2, :],
                                          ycg[64:65, QCW:2 * QCW])
                        ycgs.append(ycg)
                if J != NQC - 1:
                    # batched reciprocal of all 8 denominators for this chunk
                    recip = wk.tile([8, QCW], BF16, tag="recip")
                    with nc.allow_low_precision(reason="bf16 denom"):
                        nc.vector.reciprocal(recip[:], stage8[:])
                    for g in range(4):
                        norm_g(J, g, ycgs[g], recip, 2 * g, 2 * g + 1)
                while emitted < npieces:
                    fn, args = pieces[emitted]
                    fn(*args)
                    emitted += 1
                flush_rope()
            for fn, args in cproj_pieces(NQC - 1):
                fn(*args)

    nc.compile()
    return nc


def _get_program(sched):
    key = tuple(tuple(t) for t in sched)
    if key not in _PROG_CACHE:
        _PROG_CACHE[key] = _build_program(sched)
    return _PROG_CACHE[key]


def _prep(x, W_attn, W_proj, indices):
    import ml_dtypes
    BF = ml_dtypes.bfloat16
    half = HD // 2
    inv_freq = (1.0 / (10000.0 ** (np.arange(half, dtype=np.float32)
                                   / np.float32(half)))).astype(np.float32)

    counts = np.empty((B, T), np.int64)
    for b in range(B):
        counts[b] = np.searchsorted(indices[b], indices[b], side="right")

    sched = []
    for J in range(NQC):
        chunks = counts[:, J * QCW:(J + 1) * QCW]
        km = int((chunks.max() + PT - 1) // PT)
        tiles = []
        for i in range(km):
            lo = min(int(np.searchsorted(chunks[b], PT * i, side="right"))
                     for b in range(B))
            hi = max(int(np.searchsorted(chunks[b], PT * (i + 1) - 1,
                                         side="right"))
                     for b in range(B))
            if lo < QCW:
                tiles.append((i, 0, min(hi, QCW)))
        sched.append(tiles)

    iotas = (np.arange(PT, dtype=np.float32)[:, None]
             + PT * np.arange(NKT, dtype=np.float32)[None, :]).copy()

    # rope tables in [chan, tok] layout: freq = p mod 32, sin sign baked
    pmod = np.arange(PT) % 32
    sgn = np.where((np.arange(PT) % HD) < half, -1.0, 1.0).astype(np.float32)

    in_maps = []
    for core in range(NCORES):
        b, g = core // 2, core % 2
        wq = W_attn[:, g * QCW:(g + 1) * QCW]
        wk_ = W_attn[:, C + g * QCW: C + (g + 1) * QCW]
        wv = W_attn[:, 2 * C + g * QCW: 2 * C + (g + 1) * QCW]
        wqk = np.ascontiguousarray(
            np.concatenate([wq, wk_], axis=1)).astype(BF)
        wp = np.ascontiguousarray(W_proj[g * QCW:(g + 1) * QCW, :]).astype(BF)
        ang = inv_freq[pmod][:, None] * indices[b].astype(np.float32)[None, :]
        in_maps.append({
            "x": np.ascontiguousarray(x[b].T).astype(BF),
            "wqk": wqk,
            "wv": np.ascontiguousarray(wv).astype(BF),
            "wp": wp,
            "cosn": np.cos(ang).astype(BF),
            "sinn": (np.sin(ang) * sgn[:, None]).astype(BF),
            "cntb": np.broadcast_to(
                counts[b].astype(np.float32)[None, :], (PT, T)).copy(),
            "iotas": iotas,
        })
    return sched, in_maps


def kernel(x, W_attn, W_proj, indices):
    global _last_in_maps
    x = np.asarray(x, dtype=np.float32)
    W_attn = np.asarray(W_attn, dtype=np.float32)
    W_proj = np.asarray(W_proj, dtype=np.float32)
    indices = np.asarray(indices)

    sched, in_maps = _prep(x, W_attn, W_proj, indices)
    _last_in_maps = in_maps
    nc = _get_program(sched)

    from concourse.bass_utils import run_bass_kernel_spmd
    res = run_bass_kernel_spmd(nc, in_maps, list(range(NCORES)))

    out = np.empty((B, T, C), np.float32)
    for b in range(B):
        out[b] = res.results[2 * b]["out"] + res.results[2 * b + 1]["out"]
    return out


# revision 9
# speedup vs baseline: 1.0069x; 1.0069x over previous
"""Causal self-attention with RoPE for Trainium2, 8 NeuronCores.

Sharding: core c = (batch b = c//2, head-group g = c%2 of 8 heads).

Design (~336us on HW vs 709us for the v1 transpose-based kernel):
  - x is host-transposed to [C, T]; q,k are computed directly in
    [chan, tok] layout (W stationary, xT moving) so NO DMA transposes
    exist anywhere. v is computed in [tok, chan] layout (xT stationary,
    Wv moving) as the y-matmul needs it.
  - RoPE in [chan, tok] layout: partition-pair swap via SBUF->SBUF DMA,
    then full-width [128,512] bf16 tensor_tensor ops on DVE with
    host-prepped cos/sin tables [128, T] (sin sign baked per partition);
    the swap-dependent ops lag one block (software pipeline) so the
    swap-DMA latency never head-of-line-blocks the DVE queue.
  - exp runs on paired [128,1024] PSUM tiles (both head-halves of a key
    tile) to amortize the ~350-cycle ACT overhead; causal mask applied
    by multiplying the exp output by a 0/1 prefix mask on DVE.
  - et -> exp -> y software-pipelined by one tile so PE does not wait
    on the scalar engine (which is ~74% busy with the exps).
  - phase-1 of chunk J+1 and c_proj of earlier chunks are interleaved
    into attention of chunk J at i-step granularity; this keeps the PE
    dense so the HAM clock-gate stays at K=8/8 (2.4 GHz) instead of
    throttling to 4/8 - the single biggest effect on this kernel.
  - denominators batch-gathered per chunk into one [8,512] reciprocal
    (bf16: the 1/S error is a benign common factor per (head,query));
    per-g normalization on the last chunk shortens the serial tail.
  - c_proj in bf16 (fp32r matmuls measured ~2x slower per instruction).
"""

import numpy as np

B, T, C, H = 4, 2048, 1024, 16
HD = 64
HC = 8            # heads per core
NCORES = 8
PT = 128          # partition tile
TT = T // PT      # 16 t-tiles
QCW = 512         # q-chunk width
NQC = T // QCW    # 4
NKT = T // PT     # 16 key tiles
VW = HD + 1       # 65: v columns + ones column
CB = C // PT      # 8 c-chan blocks

_PROG_CACHE: dict = {}
_last_in_maps = None


def _build_program(sched):
    import concourse.bass as bass
    import concourse.tile as tile
    from concourse import bacc, mybir

    F32 = mybir.dt.float32
    F32R = mybir.dt.float32r
    BF16 = mybir.dt.bfloat16
    AT = mybir.ActivationFunctionType
    OP = mybir.AluOpType

    nc = bacc.Bacc("TRN2", target_bir_lowering=False, debug=False)

    x_d = nc.dram_tensor("x", [C, T], BF16, kind="ExternalInput")
    wqk_d = nc.dram_tensor("wqk", [C, 2 * QCW], BF16, kind="ExternalInput")
    wv_d = nc.dram_tensor("wv", [C, QCW], BF16, kind="ExternalInput")
    wp_d = nc.dram_tensor("wp", [QCW, C], BF16, kind="ExternalInput")
    cos_d = nc.dram_tensor("cosn", [PT, T], BF16, kind="ExternalInput")
    sin_d = nc.dram_tensor("sinn", [PT, T], BF16, kind="ExternalInput")
    cntb_d = nc.dram_tensor("cntb", [PT, T], F32, kind="ExternalInput")
    iota_d = nc.dram_tensor("iotas", [PT, NKT], F32, kind="ExternalInput")
    out_d = nc.dram_tensor("out", [T, C], F32, kind="ExternalOutput")

    with tile.TileContext(nc) as tc:
        with (
            tc.tile_pool(name="persist", bufs=1) as pp,
            tc.tile_pool(name="xtp", bufs=2) as xtp,
            tc.tile_pool(name="work", bufs=3) as wk,
            tc.tile_pool(name="ropep", bufs=3) as rp,
            tc.tile_pool(name="epool", bufs=4) as ep,
            tc.tile_pool(name="bpool", bufs=6) as bp,
            tc.tile_pool(name="ipool", bufs=3) as ip,
            tc.tile_pool(name="ycp", bufs=5) as ycp,
            tc.tile_pool(name="osb", bufs=2) as op_,
            tc.tile_pool(name="psA", bufs=2, space="PSUM") as psA,
            tc.tile_pool(name="psE", bufs=2, space="PSUM") as psE,
            tc.tile_pool(name="psY", bufs=2, space="PSUM") as psY,
        ):
            # ---------------- persistent tiles ----------------
            qT = [[pp.tile([PT, QCW], BF16, tag=f"qT{g}c{c}", name=f"qT{g}c{c}")
                   for c in range(NQC)] for g in range(4)]
            kT = [[pp.tile([PT, QCW], BF16, tag=f"kT{g}c{c}", name=f"kT{g}c{c}")
                   for c in range(NQC)] for g in range(4)]
            yTp = [[pp.tile([PT, QCW], BF16, tag=f"yTp{g}c{c}", name=f"yTp{g}c{c}")
                    for c in range(NQC)] for g in range(4)]
            vaug = [pp.tile([PT, 4 * HC * VW], BF16, tag=f"vaug{c}", name=f"vaug{c}")
                    for c in range(NQC)]
            cosT = pp.tile([PT, T], BF16, tag="cosT")
            sinT = pp.tile([PT, T], BF16, tag="sinT")
            cntb = pp.tile([PT, T], F32, tag="cntb")
            iotas = pp.tile([PT, NKT], F32, tag="iotas")
            ones = pp.tile([PT, 1], F32, tag="ones")
            w_sb = pp.tile([PT, CB * 2 * QCW], BF16, tag="w")
            wv_sb = pp.tile([PT, CB * QCW], BF16, tag="wv")
            wp_sb = pp.tile([PT, (QCW // PT) * C], BF16, tag="wp")

            # --- input DMAs: critical-path first ---
            for cb in range(CB):
                nc.sync.dma_start(
                    w_sb[:, cb * 2 * QCW:(cb + 1) * 2 * QCW],
                    wqk_d[cb * PT:(cb + 1) * PT, :],
                )
            nc.gpsimd.dma_start(
                wv_sb[:].rearrange("p (cb n) -> p cb n", n=QCW),
                wv_d[:].rearrange("(cb p) n -> p cb n", p=PT),
            )
            nc.gpsimd.dma_start(cosT[:], cos_d[:])
            nc.gpsimd.dma_start(sinT[:], sin_d[:])
            # noop
            nc.gpsimd.dma_start(cntb[:], cntb_d[:])
            nc.gpsimd.dma_start(iotas[:], iota_d[:])
            nc.sync.dma_start(
                wp_sb[:].rearrange("p (k n) -> p k n", n=C),
                wp_d[:].rearrange("(k p) n -> p k n", p=PT),
            )
            nc.vector.memset(ones[:], 1.0)
            ones_ap = ones[:]
            ones_rep = bass.AP(ones_ap.tensor, ones_ap.offset,
                               [ones_ap.ap[0], [0, 4], [0, HC]])
            for c in range(NQC):
                nc.vector.tensor_copy(
                    vaug[c][:].rearrange("p (t h c) -> p t h c", h=HC, c=VW)
                    [:, :, :, HD],
                    ones_rep,
                )

            # ---------------- phase-1 piece generators ----------------
            xts = {}
            pend_rope = []

            def flush_rope():
                while pend_rope:
                    swp, stg_c, cs_, dstT = pend_rope.pop(0)
                    tmp2 = rp.tile([PT, QCW], BF16, tag="tmp2")
                    nc.vector.tensor_tensor(tmp2[:], swp[:], sinT[:, cs_],
                                            OP.mult)
                    nc.vector.tensor_tensor(dstT[:], stg_c[:], tmp2[:],
                                            OP.add)

            def load_xt(c):
                xtc = xtp.tile([PT, CB * QCW], BF16, tag="xt")
                for hb in range(2):
                    nc.scalar.dma_start(
                        xtc[:, hb * 4 * QCW:(hb + 1) * 4 * QCW]
                        .rearrange("p (cb t) -> p cb t", t=QCW),
                        x_d[:].rearrange("(cb p) t -> p cb t", p=PT)
                        [:, 4 * hb:4 * (hb + 1), c * QCW:(c + 1) * QCW],
                    )
                xts[c] = xtc

            def p1_qk(c, m):
                """qkv block m of chunk c: 8 matmuls + rope chain."""
                cs = slice(c * QCW, (c + 1) * QCW)
                xv = xts[c][:].rearrange("p (cb t) -> p cb t", t=QCW)
                ps = psA.tile([PT, QCW], F32, tag="mm")
                for cb in range(CB):
                    nc.tensor.matmul(
                        ps[:],
                        w_sb[:, cb * 2 * QCW + m * PT:
                             cb * 2 * QCW + (m + 1) * PT],
                        xv[:, cb, :],
                        start=(cb == 0), stop=(cb == CB - 1),
                        skip_group_check=True,
                    )
                stg = rp.tile([PT, QCW], BF16, tag="stg")
                nc.vector.tensor_copy(stg[:], ps[:])
                swp = rp.tile([PT, QCW], BF16, tag="swp")
                for h in range(2):
                    b0 = h * 64
                    nc.sync.dma_start(swp[b0:b0 + 32, :],
                                      stg[b0 + 32:b0 + 64, :])
                    nc.sync.dma_start(swp[b0 + 32:b0 + 64, :],
                                      stg[b0:b0 + 32, :])
                dstT = qT[m][c] if m < 4 else kT[m - 4][c]
                tmp = rp.tile([PT, QCW], BF16, tag="tmp")
                nc.vector.tensor_tensor(tmp[:], stg[:], cosT[:, cs], OP.mult)
                pend_rope.append((swp, tmp, cs, dstT))
                if len(pend_rope) > 1:
                    swp_, stg_c, cs_, dstT_ = pend_rope.pop(0)
                    tmp2 = rp.tile([PT, QCW], BF16, tag="tmp2")
                    nc.vector.tensor_tensor(tmp2[:], swp_[:], sinT[:, cs_],
                                            OP.mult)
                    nc.vector.tensor_tensor(dstT_[:], stg_c[:], tmp2[:],
                                            OP.add)

            def p1_v(c, tt):
                """v t-tile tt of chunk c: 8 matmuls + vaug copy."""
                t = 4 * c + tt
                xv = xts[c][:].rearrange("p (cb t) -> p cb t", t=QCW)
                ps = psA.tile([PT, QCW], F32, tag="mm")
                for cb in range(CB):
                    nc.tensor.matmul(
                        ps[:],
                        xv[:, cb, tt * PT:(tt + 1) * PT],
                        wv_sb[:, cb * QCW:(cb + 1) * QCW],
                        start=(cb == 0), stop=(cb == CB - 1),
                        skip_group_check=True,
                    )
                nc.vector.tensor_copy(
                    vaug[c][:, tt * HC * VW:(tt + 1) * HC * VW]
                    .rearrange("p (h c) -> p h c", c=VW)[:, :, 0:HD],
                    ps[:].rearrange("p (h c) -> p h c", c=HD),
                )

            def p1_pieces(c):
                return ([(p1_qk, (c, m)) for m in range(8)]
                        + [(p1_v, (c, tt)) for tt in range(4)])

            # ---------------- phase 1: chunk 0 only, straight ----------------
            xtc0 = xtp.tile([PT, CB * QCW], BF16, tag="xt")
            for cb in range(CB):
                nc.scalar.dma_start(
                    xtc0[:, cb * QCW:(cb + 1) * QCW],
                    x_d[cb * PT:(cb + 1) * PT, 0:QCW],
                )
            xts[0] = xtc0
            load_xt(1)
            for m in range(8):
                p1_qk(0, m)
            for tt in range(4):
                p1_v(0, tt)
            flush_rope()

            # ---------------- phase 2+3: attention with c_proj interleave ---
            def cproj_piece(J, tt, n):
                """One c_proj output tile of chunk J."""
                t = 4 * J + tt
                ps = psA.tile([PT, QCW], F32, tag="mm")
                for k4 in range(QCW // PT):
                    nc.tensor.matmul(
                        ps[:],
                        yTp[k4][J][:, tt * PT:(tt + 1) * PT],
                        wp_sb[:, k4 * C + n * QCW:
                              k4 * C + (n + 1) * QCW],
                        start=(k4 == 0), stop=(k4 == QCW // PT - 1),
                        skip_group_check=True,
                    )
                o_sb = op_.tile([PT, QCW], F32, tag="osb")
                nc.vector.tensor_copy(o_sb[:], ps[:])
                nc.sync.dma_start(
                    out_d[t * PT:(t + 1) * PT,
                          n * QCW:(n + 1) * QCW], o_sb[:]
                )

            def cproj_pieces(J):
                return [(cproj_piece, (J, tt, n))
                        for tt in range(4) for n in range(C // QCW)]

            def norm_g(J, g, ycg, rsrc, ra, rb):
                invcA = ip.tile([64, QCW], BF16, tag="invc")
                invcB = ip.tile([64, QCW], BF16, tag="invc")
                r0A = ip.tile([1, QCW], BF16, tag="r0")
                r0B = ip.tile([1, QCW], BF16, tag="r0")
                nc.sync.dma_start(r0A[:], rsrc[ra:ra + 1, :])
                nc.sync.dma_start(r0B[:], rsrc[rb:rb + 1, :])
                nc.gpsimd.partition_broadcast(invcA[:], r0A[:])
                nc.gpsimd.partition_broadcast(invcB[:], r0B[:])
                nc.vector.tensor_tensor(
                    yTp[g][J][0:HD, :], ycg[0:HD, 0:QCW], invcA[:], OP.mult
                )
                ytmp = wk.tile([64, QCW], BF16, tag="ytmp")
                nc.vector.tensor_tensor(
                    ytmp[:], ycg[0:HD, QCW:2 * QCW], invcB[:], OP.mult
                )
                nc.sync.dma_start(yTp[g][J][HD:2 * HD, :], ytmp[:])

            for J in range(NQC):
                qs = slice(J * QCW, (J + 1) * QCW)
                # deferred PE work interleaved into this chunk's attention,
                # balanced so late (scalar-heavy) chunks get filler too
                if J == 0:
                    pieces = p1_pieces(1)
                elif J == 1:
                    pieces = p1_pieces(2) + p1_pieces(3)[:6]
                elif J == 2:
                    pieces = p1_pieces(3)[6:]
                else:
                    pieces = (cproj_pieces(0) + cproj_pieces(1)
                              + cproj_pieces(2))
                if J + 2 < NQC:
                    load_xt(J + 2)
                nsteps = len(sched[J]) * 4
                npieces = len(pieces)
                step = 0
                emitted = 0
                bts = {}
                for (i, lo, hi) in sched[J]:
                    if hi > lo:
                        bt = bp.tile([PT, QCW], BF16, tag="B")
                        nc.vector.tensor_scalar(
                            bt[:, 0:hi - lo],
                            cntb[:, J * QCW + lo:J * QCW + hi],
                            iotas[:, i:i + 1], None, OP.is_gt,
                        )
                        bts[i] = bt
                stage8 = wk.tile([8, QCW], BF16, tag="st8")
                ycgs = []
                for g in range(4):
                    yA = psY.tile([VW, QCW], F32, tag="y")
                    yB = psY.tile([VW, QCW], F32, tag="y")
                    last = sched[J][-1][0]
                    pendq = []  # software pipeline: y lags et by two tiles

                    def emit_y(p):
                        (pi, plo, pE) = p
                        for hh in range(2):
                            h = 2 * g + hh
                            vc, vk = pi // 4, pi % 4
                            vcol = vk * HC * VW + h * VW
                            ybank = yA if hh == 0 else yB
                            nc.tensor.matmul(
                                ybank[:, plo:QCW],
                                vaug[vc][:, vcol:vcol + VW],
                                pE[:, hh * QCW + plo:(hh + 1) * QCW],
                                start=(pi == sched[J][0][0]), stop=(pi == last),
                                skip_group_check=True,
                            )

                    for (i, lo, hi) in sched[J]:
                        ks = slice(i * PT, (i + 1) * PT)
                        qr = slice(J * QCW + lo, (J + 1) * QCW)
                        et = psE.tile([PT, 2 * QCW], F32, tag="et")
                        bnd = i in bts
                        kc, kk = i // 4, i % 4
                        for hh in range(2):
                            base = 64 * hh
                            nc.tensor.matmul(
                                et[:, hh * QCW + lo:(hh + 1) * QCW],
                                kT[g][kc][base:base + HD,
                                          kk * PT:(kk + 1) * PT],
                                qT[g][J][base:base + HD, lo:QCW],
                                start=True, stop=True,
                                skip_group_check=True,
                            )
                        e_sb = ep.tile([PT, 2 * QCW], BF16, tag="E")
                        ev = e_sb[:].rearrange("p (two n) -> p two n", two=2)
                        pv = et[:].rearrange("p (two n) -> p two n", two=2)
                        nc.scalar.activation(
                            ev[:, :, lo:QCW], pv[:, :, lo:QCW], AT.Exp,
                            scale=0.125,
                        )
                        if bnd:
                            btap = bts[i][:, 0:hi - lo]
                            btrep = bass.AP(btap.tensor, btap.offset,
                                            [btap.ap[0], [0, 2], btap.ap[1]])
                            nc.vector.tensor_tensor(
                                ev[:, :, lo:hi], ev[:, :, lo:hi], btrep,
                                OP.mult,
                            )
                        pendq.append((i, lo, e_sb))
                        if len(pendq) > 2:
                            emit_y(pendq.pop(0))
                        step += 1
                        while emitted * nsteps < step * npieces:
                            fn, args = pieces[emitted]
                            fn(*args)
                            emitted += 1
                    while pendq:
                        emit_y(pendq.pop(0))
                    # drain + denominator gather
                    ycg = ycp.tile([VW, 2 * QCW], BF16, tag="ycop")
                    nc.vector.tensor_copy(ycg[0:VW, 0:QCW], yA[:])
                    nc.vector.tensor_copy(ycg[0:VW, QCW:2 * QCW], yB[:])
                    if J == NQC - 1:
                        # per-g normalization: keeps the final-chunk tail
                        # chain short (only g=3's chain is serial at the end)
                        st2 = wk.tile([2, QCW], BF16, tag="st8")
                        nc.sync.dma_start(st2[0:1, :], ycg[64:65, 0:QCW])
                        nc.sync.dma_start(st2[1:2, :], ycg[64:65, QCW:2 * QCW])
                        rc2 = wk.tile([2, QCW], BF16, tag="recip")
                        with nc.allow_low_precision(reason="bf16 denom"):
                            nc.vector.reciprocal(rc2[:], st2[:])
                        norm_g(J, g, ycg, rc2, 0, 1)
                    else:
                        nc.sync.dma_start(stage8[2 * g:2 * g + 1, :],
                                          ycg[64:65, 0:QCW])
                        nc.sync.dma_start(stage8[2 * g + 1:2 * g + 2, :],
                                          ycg[64:65, QCW:2 * QCW])
                        ycgs.append(ycg)
                while emitted < npieces:
                    fn, args = pieces[emitted]
                    fn(*args)
                    emitted += 1
                flush_rope()
                if J != NQC - 1:
                    # batched reciprocal of all 8 denominators for this chunk
                    recip = wk.tile([8, QCW], BF16, tag="recip")
                    with nc.allow_low_precision(reason="bf16 denom"):
                        nc.vector.reciprocal(recip[:], stage8[:])
                    for g in range(4):
                        norm_g(J, g, ycgs[g], recip, 2 * g, 2 * g + 1)
            for fn, args in cproj_pieces(NQC - 1):
                fn(*args)

    nc.compile()
    return nc


def _get_program(sched):
    key = tuple(tuple(t) for t in sched)
    if key not in _PROG_CACHE:
        _PROG_CACHE[key] = _build_program(sched)
    return _PROG_CACHE[key]


def _prep(x, W_attn, W_proj, indices):
    import ml_dtypes
    BF = ml_dtypes.bfloat16
    half = HD // 2
    inv_freq = (1.0 / (10000.0 ** (np.arange(half, dtype=np.float32)
                                   / np.float32(half)))).astype(np.float32)

    counts = np.empty((B, T), np.int64)
    for b in range(B):
        counts[b] = np.searchsorted(indices[b], indices[b], side="right")

    sched = []
    for J in range(NQC):
        chunks = counts[:, J * QCW:(J + 1) * QCW]
        km = int((chunks.max() + PT - 1) // PT)
        tiles = []
        for i in range(km):
            lo = min(int(np.searchsorted(chunks[b], PT * i, side="right"))
                     for b in range(B))
            hi = max(int(np.searchsorted(chunks[b], PT * (i + 1) - 1,
                                         side="right"))
                     for b in range(B))
            if lo < QCW:
                tiles.append((i, 0, min(hi, QCW)))
        sched.append(tiles)

    iotas = (np.arange(PT, dtype=np.float32)[:, None]
             + PT * np.arange(NKT, dtype=np.float32)[None, :]).copy()

    # rope tables in [chan, tok] layout: freq = p mod 32, sin sign baked
    pmod = np.arange(PT) % 32
    sgn = np.where((np.arange(PT) % HD) < half, -1.0, 1.0).astype(np.float32)

    in_maps = []
    for core in range(NCORES):
        b, g = core // 2, core % 2
        wq = W_attn[:, g * QCW:(g + 1) * QCW]
        wk_ = W_attn[:, C + g * QCW: C + (g + 1) * QCW]
        wv = W_attn[:, 2 * C + g * QCW: 2 * C + (g + 1) * QCW]
        wqk = np.ascontiguousarray(
            np.concatenate([wq, wk_], axis=1)).astype(BF)
        wp = np.ascontiguousarray(W_proj[g * QCW:(g + 1) * QCW, :]).astype(BF)
        ang = inv_freq[pmod][:, None] * indices[b].astype(np.float32)[None, :]
        in_maps.append({
            "x": np.ascontiguousarray(x[b].T).astype(BF),
            "wqk": wqk,
            "wv": np.ascontiguousarray(wv).astype(BF),
            "wp": wp,
            "cosn": np.cos(ang).astype(BF),
            "sinn": (np.sin(ang) * sgn[:, None]).astype(BF),
            "cntb": np.broadcast_to(
                counts[b].astype(np.float32)[None, :], (PT, T)).copy(),
            "iotas": iotas,
        })
    return sched, in_maps


def kernel(x, W_attn, W_proj, indices):
    global _last_in_maps
    x = np.asarray(x, dtype=np.float32)
    W_attn = np.asarray(W_attn, dtype=np.float32)
    W_proj = np.asarray(W_proj, dtype=np.float32)
    indices = np.asarray(indices)

    sched, in_maps = _prep(x, W_attn, W_proj, indices)
    _last_in_maps = in_maps
    nc = _get_program(sched)

    from concourse.bass_utils import run_bass_kernel_spmd
    res = run_bass_kernel_spmd(nc, in_maps, list(range(NCORES)))

    out = np.empty((B, T, C), np.float32)
    for b in range(B):
        out[b] = res.results[2 * b]["out"] + res.results[2 * b + 1]["out"]
    return out
